# revision 7
# baseline (speedup 1.0000x reference)
"""Counter-propagation network forward pass on 8 Trainium2 NeuronCores.

Data-parallel: x sharded along batch across 8 cores; kohonen weight
table replicated. Per core: distance matmul on the PE as a 3-pass bf16
hi/lo decomposition (host passes hi/lo splits of 2*x and w; PSUM
accumulates Xhi.Whi + Xhi.Wlo + Xlo.Whi in fp32, ~2e-6 from exact
2*x.w), then s = fl(fl(mm - x2) - w2) = -d2 with the reference's fp32
rounding order, argmin via DVE max/max_index (first-occurrence
tie-break = jnp.argmin semantics on negated scores). Grossberg row
gather happens on the host during unshard (index-driven memcpy).
"""
import numpy as np
import ml_dtypes

import concourse.bacc as bacc
import concourse.bass as bass
import concourse.mybir as mybir
import concourse.tile as tile
from concourse.bass_utils import run_bass_kernel_spmd

B, D, H, DO = 32768, 256, 4096, 256
NCORES = 8
BC = B // NCORES          # 4096 rows per core
NBT = BC // 128           # 32 batch tiles per core
NHT = H // 512            # 8 H tiles of 512
dt = mybir.dt

_nc_cache = None
last_results = None

# variant knobs (sim-tuned)
CFG = dict(
    mm="bf16x3",      # "f32" | "bf16x3"
    x2_engine="pool",  # "dve" | "pool"
    w2_engine="split",  # "dve" | "pool" | "split"
    w2_dve_htiles=3,
    s_bufs=3,
)


def _build_nc(cfg=None):
    cfg = dict(CFG, **(cfg or {}))
    nc = bacc.Bacc("TRN2", target_bir_lowering=False, debug=False,
                   num_devices=NCORES)

    bf16x3 = cfg["mm"] == "bf16x3"
    if bf16x3:
        # hi/lo splits of 2*x (batch shard, transposed) and w (transposed)
        xhi = nc.dram_tensor("xhi", [D, BC], dt.bfloat16, kind="ExternalInput")
        xlo = nc.dram_tensor("xlo", [D, BC], dt.bfloat16, kind="ExternalInput")
        whi = nc.dram_tensor("whi", [D, H], dt.bfloat16, kind="ExternalInput")
        wlo = nc.dram_tensor("wlo", [D, H], dt.bfloat16, kind="ExternalInput")
    else:
        xT = nc.dram_tensor("xT", [D, BC], dt.float32, kind="ExternalInput")
        wT = nc.dram_tensor("wT", [D, H], dt.float32, kind="ExternalInput")
    w2rep = nc.dram_tensor("w2rep", [128, H], dt.float32, kind="ExternalInput")
    x2in = nc.dram_tensor("x2in", [128, NBT], dt.float32, kind="ExternalInput")

    widx = nc.dram_tensor("widx", [128, NBT], dt.uint32, kind="ExternalOutput")

    with tile.TileContext(nc) as tc:
        with (
            tc.tile_pool(name="w", bufs=1) as wpool,
            tc.tile_pool(name="s", bufs=cfg["s_bufs"]) as spool,
            tc.tile_pool(name="sm", bufs=4) as smpool,
            tc.tile_pool(name="ps", bufs=8, space="PSUM") as pspool,
        ):
            if bf16x3:
                xh = [wpool.tile([128, BC], dt.bfloat16, tag=f"xh{i}", name=f"xh{i}") for i in range(2)]
                xl = [wpool.tile([128, BC], dt.bfloat16, tag=f"xl{i}", name=f"xl{i}") for i in range(2)]
                wh = [wpool.tile([128, H], dt.bfloat16, tag=f"wh{i}", name=f"wh{i}") for i in range(2)]
                wl = [wpool.tile([128, H], dt.bfloat16, tag=f"wl{i}", name=f"wl{i}") for i in range(2)]
                for i in range(2):
                    ks = bass.ts(i, 128)
                    nc.sync.dma_start(xh[i][:], xhi[ks, :])
                    nc.sync.dma_start(xl[i][:], xlo[ks, :])
                    nc.sync.dma_start(wh[i][:], whi[ks, :])
                    nc.sync.dma_start(wl[i][:], wlo[ks, :])
            else:
                xt = [wpool.tile([128, BC], dt.float32, tag=f"xt{i}", name=f"xt{i}") for i in range(2)]
                wt = [wpool.tile([128, H], dt.float32, tag=f"wt{i}", name=f"wt{i}") for i in range(2)]
                for i in range(2):
                    ks = bass.ts(i, 128)
                    nc.sync.dma_start(xt[i][:], xT[ks, :])
                    nc.sync.dma_start(wt[i][:], wT[ks, :])
            w2t = wpool.tile([128, H], dt.float32, tag="w2t")
            x2t = wpool.tile([128, NBT], dt.float32, tag="x2t")
            wstage = wpool.tile([128, NBT], dt.uint32, tag="wstage")
            nc.sync.dma_start(w2t[:], w2rep[:, :])
            nc.sync.dma_start(x2t[:], x2in[:, :])

            x2_eng = nc.vector if cfg["x2_engine"] == "dve" else nc.gpsimd
            w2_eng = nc.vector if cfg["w2_engine"] == "dve" else nc.gpsimd

            for bt in range(NBT):
                bs = bass.ts(bt, 128)
                s = spool.tile([128, H], dt.float32, tag="s")
                for h in range(NHT):
                    hs = bass.ts(h, 512)
                    ps = pspool.tile([128, 512], dt.float32, tag="ps")
                    if bf16x3:
                        seq = [(xh[0], wh[0]), (xh[1], wh[1]),
                               (xh[0], wl[0]), (xh[1], wl[1]),
                               (xl[0], wh[0]), (xl[1], wh[1])]
                        for j, (lt, rt) in enumerate(seq):
                            nc.tensor.matmul(ps[:], lt[:, bs], rt[:, hs],
                                             start=(j == 0), stop=(j == len(seq) - 1))
                    else:
                        nc.tensor.matmul(ps[:], xt[0][:, bs], wt[0][:, hs],
                                         start=True, stop=False)
                        nc.tensor.matmul(ps[:], xt[1][:, bs], wt[1][:, hs],
                                         start=False, stop=True)
                    # evacuate mm ~= 2*x.w (exact copy)
                    nc.scalar.activation(s[:, hs], ps[:],
                                         mybir.ActivationFunctionType.Copy)
                # u = fl(mm - x2)  (exact ALU, per-partition scalar)
                x2_eng.tensor_scalar_sub(s[:], s[:], x2t[:, bt:bt + 1])
                # s = fl(u - w2)  == -d2 with reference rounding order
                if cfg["w2_engine"] == "split":
                    nd = cfg.get("w2_dve_htiles", 3)
                    cut = nd * 512
                    nc.vector.tensor_tensor(s[:, 0:cut], s[:, 0:cut],
                                            w2t[:, 0:cut],
                                            op=mybir.AluOpType.subtract)
                    nc.gpsimd.tensor_tensor(s[:, cut:H], s[:, cut:H],
                                            w2t[:, cut:H],
                                            op=mybir.AluOpType.subtract)
                else:
                    w2_eng.tensor_tensor(s[:], s[:], w2t[:],
                                         op=mybir.AluOpType.subtract)
                m8 = smpool.tile([128, 8], dt.float32, tag="m8")
                i8 = smpool.tile([128, 8], dt.uint32, tag="i8")
                nc.vector.max(m8[:], s[:])
                nc.vector.max_index(i8[:], m8[:], s[:])
                nc.vector.tensor_copy(wstage[:, bt:bt + 1], i8[:, 0:1])
            nc.sync.dma_start(widx[:, :], wstage[:, :])
    nc.compile()
    return nc


def _hi_lo(a):
    hi = a.astype(ml_dtypes.bfloat16)
    lo = (a - hi.astype(np.float32)).astype(ml_dtypes.bfloat16)
    return hi, lo


def kernel(x, kohonen_weights, grossberg_weights):
    global _nc_cache, last_results
    x = np.ascontiguousarray(x, dtype=np.float32)
    kw = np.ascontiguousarray(kohonen_weights, dtype=np.float32)
    gw = np.ascontiguousarray(grossberg_weights, dtype=np.float32)

    # x2/w2 on host with jnp-on-cpu (mirrors the reference's reduction)
    import jax
    import jax.numpy as jnp
    with jax.default_device(jax.local_devices(backend="cpu")[0]):
        x2 = np.asarray(jnp.sum(jnp.asarray(x) * jnp.asarray(x), axis=1))
        w2 = np.asarray(jnp.sum(jnp.asarray(kw) * jnp.asarray(kw), axis=1))

    w2rep = np.ascontiguousarray(np.broadcast_to(w2, (128, H)))
    base = {"w2rep": w2rep}
    if CFG["mm"] == "bf16x3":
        xT2 = np.ascontiguousarray(2.0 * x.T)           # [256, 32768], exact
        xhi_f, xlo_f = _hi_lo(xT2)
        whi_f, wlo_f = _hi_lo(np.ascontiguousarray(kw.T))
        base["whi"] = whi_f
        base["wlo"] = wlo_f
    else:
        xT2 = np.ascontiguousarray(2.0 * x.T)
        base["wT"] = np.ascontiguousarray(kw.T)

    in_maps = []
    for c in range(NCORES):
        sl = slice(c * BC, (c + 1) * BC)
        m = dict(base)
        m["x2in"] = np.ascontiguousarray(x2[sl].reshape(NBT, 128).T)
        if CFG["mm"] == "bf16x3":
            m["xhi"] = np.ascontiguousarray(xhi_f[:, sl])
            m["xlo"] = np.ascontiguousarray(xlo_f[:, sl])
        else:
            m["xT"] = np.ascontiguousarray(xT2[:, sl])
        in_maps.append(m)

    if _nc_cache is None:
        _nc_cache = _build_nc()
    res = run_bass_kernel_spmd(_nc_cache, in_maps, list(range(NCORES)))
    last_results = res

    idxs = []
    for c in range(NCORES):
        idxs.append(res.results[c]["widx"].T.reshape(BC).astype(np.int32))
    winner = np.concatenate(idxs, 0)
    output = np.ascontiguousarray(gw.T)[winner]         # [32768, 256]
    return output, winner


# revision 8
# speedup vs baseline: 1.0149x; 1.0149x over previous
"""Counter-propagation network forward pass on 8 Trainium2 NeuronCores.

Data-parallel: x sharded along batch across 8 cores; kohonen weight
table replicated. Per core: distance matmul on the PE as a 3-pass bf16
hi/lo decomposition (host passes hi/lo splits of 2*x and w; PSUM
accumulates Xhi.Whi + Xhi.Wlo + Xlo.Whi in fp32, ~2e-6 from exact
2*x.w), then s = fl(fl(mm - x2) - w2) = -d2 with the reference's fp32
rounding order, argmin via DVE max/max_index (first-occurrence
tie-break = jnp.argmin semantics on negated scores). Grossberg row
gather happens on the host during unshard (index-driven memcpy).
"""
import numpy as np
import ml_dtypes

import concourse.bacc as bacc
import concourse.bass as bass
import concourse.mybir as mybir
import concourse.tile as tile
from concourse.bass_utils import run_bass_kernel_spmd

B, D, H, DO = 32768, 256, 4096, 256
NCORES = 8
BC = B // NCORES          # 4096 rows per core
NBT = BC // 128           # 32 batch tiles per core
NHT = H // 512            # 8 H tiles of 512
dt = mybir.dt

_nc_cache = None
last_results = None

# variant knobs (sim-tuned)
CFG = dict(
    mm="bf16x3",      # "f32" | "bf16x3"
    x2_engine="pool",  # "dve" | "pool"
    w2_engine="split",  # "dve" | "pool" | "split"
    w2_dve_htiles=4,
    s_bufs=3,
)


def _build_nc(cfg=None):
    cfg = dict(CFG, **(cfg or {}))
    nc = bacc.Bacc("TRN2", target_bir_lowering=False, debug=False,
                   num_devices=NCORES)

    bf16x3 = cfg["mm"] == "bf16x3"
    if bf16x3:
        # hi/lo splits of 2*x (batch shard, transposed) and w (transposed)
        xhi = nc.dram_tensor("xhi", [D, BC], dt.bfloat16, kind="ExternalInput")
        xlo = nc.dram_tensor("xlo", [D, BC], dt.bfloat16, kind="ExternalInput")
        whi = nc.dram_tensor("whi", [D, H], dt.bfloat16, kind="ExternalInput")
        wlo = nc.dram_tensor("wlo", [D, H], dt.bfloat16, kind="ExternalInput")
    else:
        xT = nc.dram_tensor("xT", [D, BC], dt.float32, kind="ExternalInput")
        wT = nc.dram_tensor("wT", [D, H], dt.float32, kind="ExternalInput")
    w2rep = nc.dram_tensor("w2rep", [128, H], dt.float32, kind="ExternalInput")
    x2in = nc.dram_tensor("x2in", [128, NBT], dt.float32, kind="ExternalInput")

    widx = nc.dram_tensor("widx", [128, NBT], dt.uint32, kind="ExternalOutput")

    with tile.TileContext(nc) as tc:
        with (
            tc.tile_pool(name="w", bufs=1) as wpool,
            tc.tile_pool(name="s", bufs=cfg["s_bufs"]) as spool,
            tc.tile_pool(name="sm", bufs=4) as smpool,
            tc.tile_pool(name="ps", bufs=8, space="PSUM") as pspool,
        ):
            if bf16x3:
                xh = [wpool.tile([128, BC], dt.bfloat16, tag=f"xh{i}", name=f"xh{i}") for i in range(2)]
                xl = [wpool.tile([128, BC], dt.bfloat16, tag=f"xl{i}", name=f"xl{i}") for i in range(2)]
                wh = [wpool.tile([128, H], dt.bfloat16, tag=f"wh{i}", name=f"wh{i}") for i in range(2)]
                wl = [wpool.tile([128, H], dt.bfloat16, tag=f"wl{i}", name=f"wl{i}") for i in range(2)]
                for i in range(2):
                    ks = bass.ts(i, 128)
                    nc.sync.dma_start(xh[i][:], xhi[ks, :])
                    nc.sync.dma_start(xl[i][:], xlo[ks, :])
                    nc.sync.dma_start(wh[i][:], whi[ks, :])
                    nc.sync.dma_start(wl[i][:], wlo[ks, :])
            else:
                xt = [wpool.tile([128, BC], dt.float32, tag=f"xt{i}", name=f"xt{i}") for i in range(2)]
                wt = [wpool.tile([128, H], dt.float32, tag=f"wt{i}", name=f"wt{i}") for i in range(2)]
                for i in range(2):
                    ks = bass.ts(i, 128)
                    nc.sync.dma_start(xt[i][:], xT[ks, :])
                    nc.sync.dma_start(wt[i][:], wT[ks, :])
            w2t = wpool.tile([128, H], dt.float32, tag="w2t")
            x2t = wpool.tile([128, NBT], dt.float32, tag="x2t")
            wstage = wpool.tile([128, NBT], dt.uint32, tag="wstage")
            nc.sync.dma_start(w2t[:], w2rep[:, :])
            nc.sync.dma_start(x2t[:], x2in[:, :])

            x2_eng = nc.vector if cfg["x2_engine"] == "dve" else nc.gpsimd
            w2_eng = nc.vector if cfg["w2_engine"] == "dve" else nc.gpsimd

            for bt in range(NBT):
                bs = bass.ts(bt, 128)
                s = spool.tile([128, H], dt.float32, tag="s")
                for h in range(NHT):
                    hs = bass.ts(h, 512)
                    ps = pspool.tile([128, 512], dt.float32, tag="ps")
                    if bf16x3:
                        seq = [(xh[0], wh[0]), (xh[1], wh[1]),
                               (xh[0], wl[0]), (xh[1], wl[1]),
                               (xl[0], wh[0]), (xl[1], wh[1])]
                        for j, (lt, rt) in enumerate(seq):
                            nc.tensor.matmul(ps[:], lt[:, bs], rt[:, hs],
                                             start=(j == 0), stop=(j == len(seq) - 1))
                    else:
                        nc.tensor.matmul(ps[:], xt[0][:, bs], wt[0][:, hs],
                                         start=True, stop=False)
                        nc.tensor.matmul(ps[:], xt[1][:, bs], wt[1][:, hs],
                                         start=False, stop=True)
                    # evacuate mm ~= 2*x.w (exact copy)
                    nc.scalar.activation(s[:, hs], ps[:],
                                         mybir.ActivationFunctionType.Copy)
                # u = fl(mm - x2)  (exact ALU, per-partition scalar)
                x2_eng.tensor_scalar_sub(s[:], s[:], x2t[:, bt:bt + 1])
                # s = fl(u - w2)  == -d2 with reference rounding order
                if cfg["w2_engine"] == "split":
                    nd = cfg.get("w2_dve_htiles", 3)
                    cut = nd * 512
                    nc.vector.tensor_tensor(s[:, 0:cut], s[:, 0:cut],
                                            w2t[:, 0:cut],
                                            op=mybir.AluOpType.subtract)
                    nc.gpsimd.tensor_tensor(s[:, cut:H], s[:, cut:H],
                                            w2t[:, cut:H],
                                            op=mybir.AluOpType.subtract)
                else:
                    w2_eng.tensor_tensor(s[:], s[:], w2t[:],
                                         op=mybir.AluOpType.subtract)
                m8 = smpool.tile([128, 8], dt.float32, tag="m8")
                i8 = smpool.tile([128, 8], dt.uint32, tag="i8")
                nc.vector.max(m8[:], s[:])
                nc.vector.max_index(i8[:], m8[:], s[:])
                nc.vector.tensor_copy(wstage[:, bt:bt + 1], i8[:, 0:1])
            nc.sync.dma_start(widx[:, :], wstage[:, :])
    nc.compile()
    return nc


def _hi_lo(a):
    hi = a.astype(ml_dtypes.bfloat16)
    lo = (a - hi.astype(np.float32)).astype(ml_dtypes.bfloat16)
    return hi, lo


def kernel(x, kohonen_weights, grossberg_weights):
    global _nc_cache, last_results
    x = np.ascontiguousarray(x, dtype=np.float32)
    kw = np.ascontiguousarray(kohonen_weights, dtype=np.float32)
    gw = np.ascontiguousarray(grossberg_weights, dtype=np.float32)

    # x2/w2 on host with jnp-on-cpu (mirrors the reference's reduction)
    import jax
    import jax.numpy as jnp
    with jax.default_device(jax.local_devices(backend="cpu")[0]):
        x2 = np.asarray(jnp.sum(jnp.asarray(x) * jnp.asarray(x), axis=1))
        w2 = np.asarray(jnp.sum(jnp.asarray(kw) * jnp.asarray(kw), axis=1))

    w2rep = np.ascontiguousarray(np.broadcast_to(w2, (128, H)))
    base = {"w2rep": w2rep}
    if CFG["mm"] == "bf16x3":
        xT2 = np.ascontiguousarray(2.0 * x.T)           # [256, 32768], exact
        xhi_f, xlo_f = _hi_lo(xT2)
        whi_f, wlo_f = _hi_lo(np.ascontiguousarray(kw.T))
        base["whi"] = whi_f
        base["wlo"] = wlo_f
    else:
        xT2 = np.ascontiguousarray(2.0 * x.T)
        base["wT"] = np.ascontiguousarray(kw.T)

    in_maps = []
    for c in range(NCORES):
        sl = slice(c * BC, (c + 1) * BC)
        m = dict(base)
        m["x2in"] = np.ascontiguousarray(x2[sl].reshape(NBT, 128).T)
        if CFG["mm"] == "bf16x3":
            m["xhi"] = np.ascontiguousarray(xhi_f[:, sl])
            m["xlo"] = np.ascontiguousarray(xlo_f[:, sl])
        else:
            m["xT"] = np.ascontiguousarray(xT2[:, sl])
        in_maps.append(m)

    if _nc_cache is None:
        _nc_cache = _build_nc()
    res = run_bass_kernel_spmd(_nc_cache, in_maps, list(range(NCORES)))
    last_results = res

    idxs = []
    for c in range(NCORES):
        idxs.append(res.results[c]["widx"].T.reshape(BC).astype(np.int32))
    winner = np.concatenate(idxs, 0)
    output = np.ascontiguousarray(gw.T)[winner]         # [32768, 256]
    return output, winner
